# revision 1
# baseline (speedup 1.0000x reference)
"""Bilinear decoder kernel for Trainium2 (8 NeuronCores).

score_e = sigmoid(z[row_e] @ W @ z[col_e])  for 200k edges, d=512.

Strategy:
  - Shard edges across 8 cores (25000 each).
  - Per core (replicated): precompute ZW = Z @ W on the tensor engine
    (10000x512 @ 512x512) -- 20x fewer FLOPs than per-edge z1 @ W.
  - Gather ZW[row_e] and Z[col_e] rows via dma_gather, then per-edge dot
    products with the fused DVE tensor_tensor_reduce, sigmoid on ACT.

Host-side work is layout-only: index dtype/wrap conversion, a transposed
copy of z for the matmul's stationary operand, and output unshard.
"""

import sys

if "/opt/trn_rl_repo" not in sys.path:
    sys.path.insert(0, "/opt/trn_rl_repo")

from dataclasses import dataclass, field

import numpy as np


@dataclass(frozen=True)
class Cfg:
    n_cores: int = 8
    d: int = 512              # embedding dim (multiple of 128)
    n_nodes: int = 10000      # table rows
    e_total: int = 200000     # total edges
    gchunk: int = 512         # edges per dma_gather (multiple of 128).
    # Larger chunks fault the exec unit (NRT_EXEC_UNIT_UNRECOVERABLE):
    # dma_gather defaults to single_packet=True and the SDMA packet limit
    # is ~64 descriptors/engine; 512 rows = 32/engine works, 1024+ faults.
    # dtypes (numpy) for the two gather tables
    tbl_f32: bool = True      # gather tables in f32 (else bf16)
    mm_mode: str = "fp32"     # "fp32" | "fp32r" | "bf16"

    @property
    def kb(self):
        return self.d // 128

    @property
    def e_core(self):
        return self.e_total // self.n_cores

    @property
    def ep_core(self):
        # edges per core padded to a multiple of 128
        return ((self.e_core + 127) // 128) * 128

    @property
    def eblocks(self):
        return self.ep_core // 128

    @property
    def np_nodes(self):
        # node count padded to a multiple of 128
        return ((self.n_nodes + 127) // 128) * 128

    @property
    def nb(self):
        return self.np_nodes // 128

    @property
    def chunks(self):
        """List of per-gather chunk sizes (each a multiple of 128)."""
        out = []
        left = self.ep_core
        while left > 0:
            c = min(self.gchunk, left)
            out.append(c)
            left -= c
        return out


CFG = Cfg()


def build_kernel(cfg: Cfg):
    """Build + compile the Bacc module. Returns nc."""
    import concourse.bacc as bacc
    import concourse.bass as bass
    import concourse.mybir as mybir
    from concourse import tile

    f32 = mybir.dt.float32
    bf16 = mybir.dt.bfloat16
    i16 = mybir.dt.int16
    tbl_dt = f32 if cfg.tbl_f32 else bf16
    if cfg.mm_mode == "fp32":
        mm_dt = f32
    elif cfg.mm_mode == "fp32r":
        mm_dt = mybir.dt.float32r
    else:
        mm_dt = bf16

    D, KB, NP, NB = cfg.d, cfg.kb, cfg.np_nodes, cfg.nb
    idx_cols = cfg.ep_core // 16

    nc = bacc.Bacc(
        "TRN2", target_bir_lowering=False, debug=False, num_devices=cfg.n_cores
    )

    # matmul operands are in mm dtype; gather tables in tbl dtype
    zt = nc.dram_tensor("zt", [D, NP], mm_dt, kind="ExternalInput")
    ztbl = nc.dram_tensor("ztbl", [cfg.n_nodes, D], tbl_dt, kind="ExternalInput")
    w = nc.dram_tensor("w", [D, D], mm_dt, kind="ExternalInput")
    ridx = nc.dram_tensor("ridx", [128, idx_cols], i16, kind="ExternalInput")
    cidx = nc.dram_tensor("cidx", [128, idx_cols], i16, kind="ExternalInput")
    scores = nc.dram_tensor("scores", [128, cfg.eblocks], f32, kind="ExternalOutput")
    zw = nc.dram_tensor("zw", [NP, D], tbl_dt)  # internal

    with tile.TileContext(nc) as tc:
        with (
            tc.tile_pool(name="const", bufs=1) as constp,
            tc.tile_pool(name="ztp", bufs=3) as ztp,
            tc.tile_pool(name="zwsb", bufs=2) as zwsb,
            tc.tile_pool(name="rows", bufs=2) as rowsp,
            tc.tile_pool(name="cols", bufs=2) as colsp,
            tc.tile_pool(name="prod", bufs=4) as prodp,
            tc.tile_pool(name="ps", bufs=4, space="PSUM") as psp,
        ):
            # ---- constants ----
            w_sb = constp.tile([128, KB, D], mm_dt, tag="w")
            nc.sync.dma_start(w_sb[:], w.ap().rearrange("(kb p) f -> p kb f", p=128))
            ridx_sb = constp.tile([128, idx_cols], i16, tag="ridx")
            nc.sync.dma_start(ridx_sb[:], ridx.ap())
            cidx_sb = constp.tile([128, idx_cols], i16, tag="cidx")
            nc.sync.dma_start(cidx_sb[:], cidx.ap())
            scores_sb = constp.tile([128, cfg.eblocks], f32, tag="scores")
            sig_sb = constp.tile([128, cfg.eblocks], f32, tag="sig")
            scratch = constp.tile([128, D], f32, tag="scratch")

            zt_r = zt.ap().rearrange("(kb p) n -> p kb n", p=128)

            # ---- phase 1: ZW = Z @ W ----
            for nb in range(NB):
                zt_t = ztp.tile([128, KB, 128], mm_dt, tag="zt")
                nc.sync.dma_start(zt_t[:], zt_r[:, :, nb * 128 : (nb + 1) * 128])
                ps = psp.tile([128, D], f32, tag="ps")
                for kb in range(KB):
                    nc.tensor.matmul(
                        ps[:],
                        lhsT=zt_t[:, kb, :],
                        rhs=w_sb[:, kb, :],
                        start=(kb == 0),
                        stop=(kb == KB - 1),
                    )
                zw_t = zwsb.tile([128, D], tbl_dt, tag="zwt")
                nc.vector.tensor_copy(zw_t[:], ps[:])
                nc.sync.dma_start(zw[nb * 128 : (nb + 1) * 128, :], zw_t[:])

            # ---- phase 2: gathers + per-edge dots ----
            blk = 0  # global 128-edge block counter
            off = 0  # idx column offset
            for ci, G in enumerate(cfg.chunks):
                gb = G // 128
                ctile = colsp.tile([128, cfg.gchunk // 128, D], tbl_dt, tag="ct")
                nc.gpsimd.dma_gather(
                    ctile[:, :gb, :],
                    ztbl.ap(),
                    cidx_sb[:, off : off + G // 16],
                    num_idxs=G,
                    num_idxs_reg=G,
                    elem_size=D,
                )
                rtile = rowsp.tile([128, cfg.gchunk // 128, D], tbl_dt, tag="rt")
                nc.gpsimd.dma_gather(
                    rtile[:, :gb, :],
                    zw.ap(),
                    ridx_sb[:, off : off + G // 16],
                    num_idxs=G,
                    num_idxs_reg=G,
                    elem_size=D,
                )
                for b in range(gb):
                    # DVE multiply, then ACT copy-with-accumulate = free-dim sum.
                    # (tensor_tensor_reduce is unsupported by this runtime.)
                    prod = prodp.tile([128, D], f32, tag="prod")
                    nc.vector.tensor_mul(prod[:], rtile[:, b, :], ctile[:, b, :])
                    nc.scalar.activation(
                        scratch[:],
                        prod[:],
                        mybir.ActivationFunctionType.Copy,
                        accum_out=scores_sb[:, blk : blk + 1],
                    )
                    blk += 1
                off += G // 16

            # ---- sigmoid + writeback ----
            nc.scalar.activation(
                sig_sb[:], scores_sb[:], mybir.ActivationFunctionType.Sigmoid
            )
            nc.sync.dma_start(scores.ap(), sig_sb[:])

    nc.compile()
    return nc


def _wrap_idx(ids: np.ndarray, cfg: Cfg) -> np.ndarray:
    """int node-ids [ep_core] -> [128, ep_core//16] int16 in the 16-partition
    wrapped layout dma_gather expects (replicated across the 8 Q7 cores)."""
    out = np.empty((16, cfg.ep_core // 16), dtype=np.int16)
    off = 0
    for G in cfg.chunks:
        c = ids[off : off + G].reshape(G // 16, 16).T  # [16, G/16]
        out[:, off // 16 : (off + G) // 16] = c
        off += G
    return np.tile(out, (8, 1))


def prep_inputs(z_drug, weight, batch_edges, cfg: Cfg):
    """Host-side layout prep. Returns (shared_map, per_core_maps)."""
    z = np.ascontiguousarray(np.asarray(z_drug, dtype=np.float32))
    w = np.ascontiguousarray(np.asarray(weight, dtype=np.float32))
    be = np.asarray(batch_edges)

    mm_np = np.float32  # zt/w host dtype for fp32 and fp32r modes
    if cfg.mm_mode == "bf16":
        import ml_dtypes

        mm_np = ml_dtypes.bfloat16
    tbl_np = np.float32
    if not cfg.tbl_f32:
        import ml_dtypes

        tbl_np = ml_dtypes.bfloat16

    zt = np.zeros((cfg.d, cfg.np_nodes), dtype=mm_np)
    zt[:, : cfg.n_nodes] = z.T.astype(mm_np)
    ztbl = np.ascontiguousarray(z.astype(tbl_np))
    w_in = np.ascontiguousarray(w.astype(mm_np))

    shared = {"zt": zt, "ztbl": ztbl, "w": w_in}
    per_core = []
    for c in range(cfg.n_cores):
        sl = slice(c * cfg.e_core, (c + 1) * cfg.e_core)
        rids = np.zeros(cfg.ep_core, dtype=np.int64)
        cids = np.zeros(cfg.ep_core, dtype=np.int64)
        rids[: cfg.e_core] = be[0, sl]
        cids[: cfg.e_core] = be[1, sl]
        per_core.append(
            {"ridx": _wrap_idx(rids, cfg), "cidx": _wrap_idx(cids, cfg)}
        )
    return shared, per_core


_NC_CACHE = {}


def get_nc(cfg: Cfg):
    key = (cfg.tbl_f32, cfg.mm_mode, cfg.gchunk)
    if key not in _NC_CACHE:
        _NC_CACHE[key] = build_kernel(cfg)
    return _NC_CACHE[key]


class Runner:
    """Reusable jitted multi-core runner (mirrors bass2jax.run_bass_via_pjrt's
    n_cores>1 path) so repeated calls don't retrace/recompile."""

    def __init__(self, cfg: Cfg):
        import jax
        import concourse.mybir as mybir
        from concourse import bass2jax
        from concourse.bass2jax import _bass_exec_p, partition_id_tensor
        from jax.experimental.shard_map import shard_map
        from jax.sharding import Mesh, PartitionSpec

        bass2jax.install_neuronx_cc_hook()
        nc = get_nc(cfg)
        self.cfg = cfg
        self.nc = nc
        self.jax = jax

        in_names, out_names, out_avals, zero_outs = [], [], [], []
        for alloc in nc.m.functions[0].allocations:
            if not isinstance(alloc, mybir.MemoryLocationSet):
                continue
            name = alloc.memorylocations[0].name
            if alloc.kind == "ExternalInput":
                in_names.append(name)
            elif alloc.kind == "ExternalOutput":
                out_names.append(name)
                shape = tuple(alloc.tensor_shape)
                dtype = mybir.dt.np(alloc.dtype)
                out_avals.append(jax.core.ShapedArray(shape, dtype))
                zero_outs.append(np.zeros(shape, dtype))
        partition_name = (
            nc.partition_id_tensor.name if nc.partition_id_tensor else None
        )
        if partition_name is not None:
            in_names.remove(partition_name)
        n_params = len(in_names)
        in_names = in_names + out_names
        if partition_name is not None:
            in_names.append(partition_name)
        self.in_names, self.out_names = in_names, out_names
        self.out_avals, self.zero_outs = out_avals, zero_outs
        self.n_params = n_params

        def _body(*args):
            operands = list(args)
            if partition_name is not None:
                operands.append(partition_id_tensor())
            outs = _bass_exec_p.bind(
                *operands,
                out_avals=tuple(out_avals),
                in_names=tuple(in_names),
                out_names=tuple(out_names),
                lowering_input_output_aliases=(),
                sim_require_finite=True,
                sim_require_nnan=True,
                nc=nc,
            )
            return tuple(outs)

        n_outs = len(out_names)
        donate = tuple(range(n_params, n_params + n_outs))
        devices = jax.devices()[: cfg.n_cores]
        self.mesh = Mesh(np.asarray(devices), ("core",))
        self.sharding = jax.sharding.NamedSharding(
            self.mesh, PartitionSpec("core")
        )
        in_specs = (PartitionSpec("core"),) * (n_params + n_outs)
        out_specs = (PartitionSpec("core"),) * n_outs
        self.sharded = jax.jit(
            shard_map(
                _body,
                mesh=self.mesh,
                in_specs=in_specs,
                out_specs=out_specs,
                check_rep=False,
            ),
            donate_argnums=donate,
            keep_unused=True,
        )
        # Identity jit used to place host arrays on-device with the right
        # sharding via the same transfer path the kernel call uses (raw
        # device_put with NamedSharding desyncs the axon mesh).
        self.loader = jax.jit(
            lambda a: a, in_shardings=self.sharding, out_shardings=self.sharding
        )
        self.concat_in_dev = None

    def set_inputs(self, in_maps):
        import jax

        n = self.cfg.n_cores
        concat_in = [
            np.concatenate(
                [np.asarray(in_maps[c][name]) for c in range(n)], axis=0
            )
            for name in self.in_names[: self.n_params]
        ]
        self.concat_in_dev = [self.loader(a) for a in concat_in]
        for a in self.concat_in_dev:
            a.block_until_ready()

    def call(self):
        """One execution; returns (per-core results, wall seconds)."""
        import time

        n = self.cfg.n_cores
        zeros = [
            np.zeros((n * z.shape[0], *z.shape[1:]), z.dtype)
            for z in self.zero_outs
        ]
        zeros_dev = [self.loader(z) for z in zeros]
        for z in zeros_dev:
            z.block_until_ready()
        t0 = time.perf_counter()
        out_arrs = self.sharded(*self.concat_in_dev, *zeros_dev)
        for o in out_arrs:
            o.block_until_ready()
        wall = time.perf_counter() - t0
        results = [
            {
                name: np.asarray(out_arrs[i]).reshape(
                    n, *self.out_avals[i].shape
                )[c]
                for i, name in enumerate(self.out_names)
            }
            for c in range(n)
        ]
        return results, wall


_RUNNER_CACHE = {}


def get_runner(cfg: Cfg) -> Runner:
    key = (cfg.tbl_f32, cfg.mm_mode, cfg.gchunk)
    if key not in _RUNNER_CACHE:
        _RUNNER_CACHE[key] = Runner(cfg)
    return _RUNNER_CACHE[key]


def _unshard(results, cfg: Cfg) -> np.ndarray:
    parts = []
    for c in range(cfg.n_cores):
        raw = results[c]["scores"]  # [128, eblocks], edge i at [i%128, i//128]
        parts.append(raw.T.reshape(-1)[: cfg.e_core])
    return np.concatenate(parts).astype(np.float32)


def run(z_drug, weight, batch_edges, cfg: Cfg, repeats: int = 1):
    """Returns (scores[200000] f32, [wall seconds per call]).

    Uses the plain run_bass_kernel_spmd path (numpy inputs, fresh jit per
    call). The fancier resident-input Runner desyncs the axon mesh, so walls
    here include input-transfer + dispatch overhead.
    """
    import time

    from concourse.bass_utils import run_bass_kernel_spmd

    nc = get_nc(cfg)
    shared, per_core = prep_inputs(z_drug, weight, batch_edges, cfg)
    in_maps = [dict(shared, **pc) for pc in per_core]
    walls = []
    res = None
    for _ in range(max(1, repeats)):
        t0 = time.perf_counter()
        try:
            res = run_bass_kernel_spmd(
                nc, in_maps, core_ids=list(range(cfg.n_cores))
            )
        except Exception:
            if res is not None:
                break  # keep earlier good result; a repeat run hiccupped
            time.sleep(30)
            res = run_bass_kernel_spmd(
                nc, in_maps, core_ids=list(range(cfg.n_cores))
            )
        walls.append(time.perf_counter() - t0)
    return _unshard(res.results, cfg), walls


def kernel(z_drug, weight, batch_edges):
    out, _ = run(z_drug, weight, batch_edges, CFG)
    return out



# revision 2
# speedup vs baseline: 13.0746x; 13.0746x over previous
"""Bilinear decoder kernel for Trainium2 (8 NeuronCores).

score_e = sigmoid(z[row_e] @ W @ z[col_e])  for 200k edges, d=512.

v2 strategy (host->device transfer over axon is ~40MB/s, so uploads are
sharded and the tables are rebuilt on-device with AllGathers):
  - Upload per core: z shard [1280,512] bf16 (1/8 of nodes), W shard
    [64,512] bf16 (1/8 of rows), edge indices [16, 2*1568] int16.
    ~1.6MB/core vs ~41MB/core for the replicated-f32 baseline.
  - Device: AllGather W (tiny) -> full W in SBUF. Load z^T via
    dma_start_transpose, matmul ZW_c = z_c @ W for the local 1280-node
    shard (tensor engine, bf16). AllGather z -> full Z table [10240,512]
    in DRAM; AllGather ZW_c -> full ZW table.
  - Gather ZW[row_e] and Z[col_e] rows via dma_gather, per-edge dot via
    DVE mul + ACT copy-with-accumulate, sigmoid on ACT.
  - Edges sharded 25000/core; node ids are remapped on host to the
    padded AllGather layout (node n -> (n//1250)*1280 + n%1250).

Host-side work is layout-only: bf16 casts, shard slicing, index
wrap/remap, output unshard.
"""

import sys

if "/opt/trn_rl_repo" not in sys.path:
    sys.path.insert(0, "/opt/trn_rl_repo")

from dataclasses import dataclass

import numpy as np


@dataclass(frozen=True)
class Cfg:
    n_cores: int = 8
    d: int = 512              # embedding dim
    n_nodes: int = 10000      # node table rows
    e_total: int = 200000     # total edges
    gchunk: int = 512         # edges per dma_gather (SDMA packet limit:
    #                           512 rows = 32 descriptors/engine works,
    #                           1024+ faults the exec unit)

    @property
    def kb(self):
        return self.d // 128  # 4

    @property
    def nsh(self):
        return self.n_nodes // self.n_cores  # 1250 nodes per core

    @property
    def nshp(self):
        return ((self.nsh + 127) // 128) * 128  # 1280 padded

    @property
    def nblocks(self):
        return self.nshp // 128  # 10

    @property
    def ntab(self):
        return self.nshp * self.n_cores  # 10240 table rows

    @property
    def wsh(self):
        return self.d // self.n_cores  # 64 W rows per core

    @property
    def e_core(self):
        return self.e_total // self.n_cores  # 25000

    @property
    def ep_core(self):
        return ((self.e_core + 127) // 128) * 128  # 25088

    @property
    def eblocks(self):
        return self.ep_core // 128  # 196

    @property
    def idx_cols(self):
        return self.ep_core // 16  # 1568

    @property
    def chunks(self):
        out = []
        left = self.ep_core
        while left > 0:
            c = min(self.gchunk, left)
            out.append(c)
            left -= c
        return out


CFG = Cfg()


def build_kernel(cfg: Cfg):
    """Build + compile the Bacc module. Returns nc."""
    import concourse.bacc as bacc
    import concourse.mybir as mybir
    from concourse import tile

    f32 = mybir.dt.float32
    bf16 = mybir.dt.bfloat16
    i16 = mybir.dt.int16

    D, KB, NSHP, NB = cfg.d, cfg.kb, cfg.nshp, cfg.nblocks
    NTAB, WSH, IC = cfg.ntab, cfg.wsh, cfg.idx_cols
    group = [list(range(cfg.n_cores))]

    nc = bacc.Bacc(
        "TRN2", target_bir_lowering=False, debug=False, num_devices=cfg.n_cores
    )

    zin = nc.dram_tensor("zin", [NSHP, D], bf16, kind="ExternalInput")
    win = nc.dram_tensor("win", [WSH, D], bf16, kind="ExternalInput")
    eidx = nc.dram_tensor("eidx", [16, 2 * IC], i16, kind="ExternalInput")
    scores = nc.dram_tensor("scores", [128, cfg.eblocks], f32, kind="ExternalOutput")

    with tile.TileContext(nc) as tc:
        with (
            tc.tile_pool(name="const", bufs=1) as constp,
            tc.tile_pool(name="dram", bufs=1, space="DRAM") as dramp,
            tc.tile_pool(name="zwsb", bufs=2) as zwp,
            tc.tile_pool(name="rows", bufs=2) as rowsp,
            tc.tile_pool(name="cols", bufs=2) as colsp,
            tc.tile_pool(name="prod", bufs=4) as prodp,
            tc.tile_pool(name="ps", bufs=2, space="PSUM") as psp,
        ):
            # ---- DRAM bounce buffers (collectives can't touch I/O tensors) ----
            wag_in = dramp.tile([WSH, D], bf16, tag="wag_in")
            wag_out = dramp.tile([D, D], bf16, tag="wag_out")
            zag_in = dramp.tile([NSHP, D], bf16, tag="zag_in")
            zag_out = dramp.tile([NTAB, D], bf16, tag="zag_out")
            zwag_in = dramp.tile([NSHP, D], bf16, tag="zwag_in")
            zwag_out = dramp.tile([NTAB, D], bf16, tag="zwag_out")

            nc.sync.dma_start(wag_in[:], win.ap())
            nc.sync.dma_start(zag_in[:], zin.ap())

            # ---- collectives (gpsimd, straight-line order) ----
            nc.gpsimd.collective_compute(
                "AllGather",
                mybir.AluOpType.bypass,
                replica_groups=group,
                ins=[wag_in.opt()],
                outs=[wag_out.opt()],
            )
            nc.gpsimd.collective_compute(
                "AllGather",
                mybir.AluOpType.bypass,
                replica_groups=group,
                ins=[zag_in.opt()],
                outs=[zag_out.opt()],
            )

            # ---- SBUF constants ----
            w_sb = constp.tile([128, KB, D], bf16, tag="w")
            nc.sync.dma_start(
                w_sb[:], wag_out[:].rearrange("(kb p) f -> p kb f", p=128)
            )
            # z^T for the matmul's stationary operand: [128, kb, NSHP]
            zt_sb = constp.tile([128, KB, NSHP], bf16, tag="zt")
            nc.sync.dma_start_transpose(zt_sb[:], zin.ap())
            # edge indices: upload 16-partition wrap, replicate to 128
            idx_sb = constp.tile([128, 2 * IC], i16, tag="idx")
            nc.sync.dma_start(idx_sb[0:16, :], eidx.ap())
            for r in range(1, 8):
                nc.sync.dma_start(idx_sb[16 * r : 16 * (r + 1), :], idx_sb[0:16, :])
            scores_sb = constp.tile([128, cfg.eblocks], f32, tag="scores")
            sig_sb = constp.tile([128, cfg.eblocks], f32, tag="sig")
            scratch = constp.tile([128, D], f32, tag="scratch")

            # ---- phase 1: ZW_c = z_c @ W for the local node shard ----
            for nb in range(NB):
                ps = psp.tile([128, D], f32, tag="ps")
                for kb in range(KB):
                    nc.tensor.matmul(
                        ps[:],
                        lhsT=zt_sb[:, kb, nb * 128 : (nb + 1) * 128],
                        rhs=w_sb[:, kb, :],
                        start=(kb == 0),
                        stop=(kb == KB - 1),
                    )
                zw_t = zwp.tile([128, D], bf16, tag="zwt")
                nc.vector.tensor_copy(zw_t[:], ps[:])
                nc.sync.dma_start(zwag_in[nb * 128 : (nb + 1) * 128, :], zw_t[:])

            nc.gpsimd.collective_compute(
                "AllGather",
                mybir.AluOpType.bypass,
                replica_groups=group,
                ins=[zwag_in.opt()],
                outs=[zwag_out.opt()],
            )

            # ---- phase 2: gathers + per-edge dots ----
            blk = 0
            off = 0
            for G in cfg.chunks:
                gb = G // 128
                ctile = colsp.tile([128, cfg.gchunk // 128, D], bf16, tag="ct")
                nc.gpsimd.dma_gather(
                    ctile[:, :gb, :],
                    zag_out[:],
                    idx_sb[:, IC + off : IC + off + G // 16],
                    num_idxs=G,
                    num_idxs_reg=G,
                    elem_size=D,
                )
                rtile = rowsp.tile([128, cfg.gchunk // 128, D], bf16, tag="rt")
                nc.gpsimd.dma_gather(
                    rtile[:, :gb, :],
                    zwag_out[:],
                    idx_sb[:, off : off + G // 16],
                    num_idxs=G,
                    num_idxs_reg=G,
                    elem_size=D,
                )
                for b in range(gb):
                    # DVE multiply, then ACT copy-with-accumulate = free-dim sum.
                    prod = prodp.tile([128, D], f32, tag="prod")
                    nc.vector.tensor_mul(prod[:], rtile[:, b, :], ctile[:, b, :])
                    nc.scalar.activation(
                        scratch[:],
                        prod[:],
                        mybir.ActivationFunctionType.Copy,
                        accum_out=scores_sb[:, blk : blk + 1],
                    )
                    blk += 1
                off += G // 16

            # ---- sigmoid + writeback ----
            nc.scalar.activation(
                sig_sb[:], scores_sb[:], mybir.ActivationFunctionType.Sigmoid
            )
            nc.sync.dma_start(scores.ap(), sig_sb[:])

    nc.compile()
    return nc


def _wrap_idx(ids: np.ndarray, cfg: Cfg) -> np.ndarray:
    """int table-row ids [ep_core] -> [16, ep_core//16] int16 in the
    16-partition wrapped layout dma_gather expects."""
    out = np.empty((16, cfg.ep_core // 16), dtype=np.int16)
    off = 0
    for G in cfg.chunks:
        c = ids[off : off + G].reshape(G // 16, 16).T  # [16, G/16]
        out[:, off // 16 : (off + G) // 16] = c
        off += G
    return out


def prep_inputs(z_drug, weight, batch_edges, cfg: Cfg):
    """Host-side layout prep. Returns per-core input maps."""
    import ml_dtypes

    bf16 = ml_dtypes.bfloat16

    z = np.asarray(z_drug, dtype=np.float32)
    w = np.asarray(weight, dtype=np.float32)
    be = np.asarray(batch_edges)

    per_core = []
    for c in range(cfg.n_cores):
        # z shard: nodes [c*1250, (c+1)*1250), padded to 1280 rows
        zsh = np.zeros((cfg.nshp, cfg.d), dtype=bf16)
        zsh[: cfg.nsh] = z[c * cfg.nsh : (c + 1) * cfg.nsh].astype(bf16)
        # W shard: rows [c*64, (c+1)*64)
        wsh = np.ascontiguousarray(
            w[c * cfg.wsh : (c + 1) * cfg.wsh].astype(bf16)
        )
        # edge shard + remap node ids to the padded AllGather table layout
        sl = slice(c * cfg.e_core, (c + 1) * cfg.e_core)
        rids = np.zeros(cfg.ep_core, dtype=np.int64)
        cids = np.zeros(cfg.ep_core, dtype=np.int64)
        rids[: cfg.e_core] = be[0, sl]
        cids[: cfg.e_core] = be[1, sl]
        rids = (rids // cfg.nsh) * cfg.nshp + rids % cfg.nsh
        cids = (cids // cfg.nsh) * cfg.nshp + cids % cfg.nsh
        eidx = np.concatenate(
            [_wrap_idx(rids, cfg), _wrap_idx(cids, cfg)], axis=1
        )
        per_core.append({"zin": zsh, "win": wsh, "eidx": eidx})
    return per_core


_NC_CACHE = {}


def get_nc(cfg: Cfg):
    key = (cfg.gchunk,)
    if key not in _NC_CACHE:
        _NC_CACHE[key] = build_kernel(cfg)
    return _NC_CACHE[key]


def _unshard(results, cfg: Cfg) -> np.ndarray:
    parts = []
    for c in range(cfg.n_cores):
        raw = results[c]["scores"]  # [128, eblocks], edge i at [i%128, i//128]
        parts.append(raw.T.reshape(-1)[: cfg.e_core])
    return np.concatenate(parts).astype(np.float32)


def run(z_drug, weight, batch_edges, cfg: Cfg, repeats: int = 1):
    """Returns (scores[200000] f32, [wall seconds per call])."""
    import time

    from concourse.bass_utils import run_bass_kernel_spmd

    nc = get_nc(cfg)
    in_maps = prep_inputs(z_drug, weight, batch_edges, cfg)
    walls = []
    res = None
    for _ in range(max(1, repeats)):
        t0 = time.perf_counter()
        try:
            res = run_bass_kernel_spmd(
                nc, in_maps, core_ids=list(range(cfg.n_cores))
            )
        except Exception:
            if res is not None:
                break  # keep earlier good result; a repeat run hiccupped
            time.sleep(30)
            res = run_bass_kernel_spmd(
                nc, in_maps, core_ids=list(range(cfg.n_cores))
            )
        walls.append(time.perf_counter() - t0)
    return _unshard(res.results, cfg), walls


def kernel(z_drug, weight, batch_edges):
    out, _ = run(z_drug, weight, batch_edges, CFG)
    return out
